# revision 25
# baseline (speedup 1.0000x reference)
"""Trainium2 Bass kernel for the CMB power-spectrum emulator problem.

Math: a 4-layer MLP maps phi (512,2) -> diag (128 knots, 512 ch); a natural
cubic spline through the 128 knots is evaluated on a constant 256x256
isotropic-frequency grid, then exp(.)*NORM.

Structural collapses (all input-independent or host-cheap):
 1. The spline is linear in the knot values: the whole spline stage is a
    constant matrix E (grid_points, 128) applied to the knot values.
 2. The grid has an exact 8-fold dihedral symmetry: only 8385 of the 65536
    grid points are distinct; the device computes the unique points and the
    host replicates them with a constant gather.
 3. The last (linear) MLP layer commutes with E: G = E @ W4.T (P,100) and
    c = E @ b4 + ln NORM are folded on the host, so the device runs only the
    3 relu layers and one (102-row) matmul per point block. The per-point
    bias c rides inside the matmul as two bf16 hi/lo rows multiplied by
    constant-1 rows of the stationary operand (bf16 alone would lose the
    ~9.45 magnitude of ln NORM; the hi/lo split restores ~16-bit precision).

Device work per core (point sharding, 1056 points, 512 channels, all bf16):
  junk matmuls warm the PE HAM clock-gate during the input-DMA window;
  a dummy exp preloads the ACT table set at t=0.
  MLP as two interleaved 256-wide chains -> h3 (100, 512) bf16 (+ ones rows)
  per 128-channel group g: psum = h3_g.T @ GT   (TensorE, bf16, K=102)
                           stage = exp(psum)    (ScalarE LUT)
                           store (128, 1056) bf16 (DMA, issue spread over
                           sync/gpsimd/vector queues)
Host: upcast bf16->f32, constant gather to (512, 256, 256).
"""

import os

import ml_dtypes
import numpy as np

B = 512
N_CORES = 8
N_UNIQ = 129 * 130 // 2       # 8385 distinct grid values
P_CORE = 1056                 # per-core unique points (8 x 1056 = 8448 padded)
P_PAD = N_CORES * P_CORE
NORM = 1.0 / 12661.0
HB = B // 2                   # per-chain batch width

MIN_PHI = np.array([50.0, 0.0075], np.float64)
DPHI = np.array([40.0, 0.0492], np.float64)
MU = np.array([70.0, 0.032], np.float64)
SIG = np.array([20.0, 0.025], np.float64)

PA_COLS = 612   # phiT (512) | W1f (100); row 2 = ones | b1f (bias fold)
PA_ROWS = 3
GT_ROWS = 102   # G.T (100) ; c_hi ; c_lo (+ ones rows in lhsT)
GA_COLS = 100 + 512   # W2aug | G.T cols 0:512 (gates L2: lands first)
GB_COLS = 100 + P_CORE - 512  # W3aug | G.T cols 512:1056

_CACHE = {}


def _bf16(x):
    return np.asarray(x, np.float64).astype(ml_dtypes.bfloat16)


def _spline_eval_matrix_f64(wn_vals):
    """E (len(wn_vals), 128) f64: natural-cubic-spline evaluation at wn_vals,
    linear in the 128 knot values (knots t_k = sqrt(2)*k in fp32)."""
    wn = (256.0 * np.fft.fftfreq(256, d=1.0)).reshape(256, 1)
    wn_iso = np.sqrt(wn**2 + wn.reshape(1, 256) ** 2)
    t32 = np.fft.fftshift(wn_iso).diagonal()[128:].astype(np.float32)  # (128,)

    n = 128
    t = t32.astype(np.float64)
    h = np.diff(t)
    A = np.diag(2.0 * (h[:-1] + h[1:])) + np.diag(h[1:-1], 1) + np.diag(h[1:-1], -1)
    D1 = np.zeros((n - 1, n))
    for i in range(n - 1):
        D1[i, i] = -1.0 / h[i]
        D1[i, i + 1] = 1.0 / h[i]
    D2 = 6.0 * (D1[1:] - D1[:-1])
    L = np.zeros((n, n))
    L[1:-1] = np.linalg.solve(A, D2)

    Sa = np.eye(n)[: n - 1]
    Sb = D1 - (h[:, None] / 6.0) * (2.0 * L[:-1] + L[1:])
    Sc = L[:-1] / 2.0
    Sd = (L[1:] - L[:-1]) / (6.0 * h[:, None])

    w32 = wn_vals.astype(np.float32)
    idx = np.clip(np.searchsorted(t32, w32, side="right") - 1, 0, n - 2)
    f = (w32 - t32[idx]).astype(np.float64)[:, None]
    return Sa[idx] + f * (Sb[idx] + f * (Sc[idx] + f * Sd[idx]))  # f64


def _build_constants():
    """E64 (P_PAD, 128) f64 spline-eval matrix at the unique points (zero
    rows for padding), and IDX (65536,) int32 full-grid -> unique column."""
    k = np.arange(256)
    absw = np.minimum(k, 256 - k)
    ai = np.minimum(absw[:, None], absw[None, :])
    bi = np.maximum(absw[:, None], absw[None, :])
    uid = (bi * (bi + 1)) // 2 + ai  # (256,256) in [0, N_UNIQ)

    bs = np.concatenate([np.full(b + 1, b) for b in range(129)])
    as_ = np.concatenate([np.arange(b + 1) for b in range(129)])
    wn_vals = np.sqrt(as_.astype(np.float64) ** 2 + bs.astype(np.float64) ** 2)

    E = np.zeros((P_PAD, 128), np.float64)
    E[:N_UNIQ] = _spline_eval_matrix_f64(wn_vals)
    return E, uid.ravel().astype(np.int32)


def _build_program():
    import concourse.bass as bass
    import concourse.bacc as bacc
    import concourse.mybir as mybir
    from concourse import tile

    f32 = mybir.dt.float32
    bf16 = mybir.dt.bfloat16
    nc = bacc.Bacc("TRN2", target_bir_lowering=False, debug=False)

    pa_d = nc.dram_tensor("pa", [PA_ROWS, PA_COLS], bf16, kind="ExternalInput")
    ga_d = nc.dram_tensor("ga", [GT_ROWS, GA_COLS], bf16, kind="ExternalInput")
    gb_d = nc.dram_tensor("gb", [GT_ROWS, GB_COLS], bf16, kind="ExternalInput")
    out_d = nc.dram_tensor("out", [B, P_CORE], bf16, kind="ExternalOutput")

    Exp = mybir.ActivationFunctionType.Exp
    Relu = mybir.ActivationFunctionType.Relu
    Max = mybir.AluOpType.max

    N_GRP = 4
    SUB = 512  # matmul free chunk (PSUM bank)

    with tile.TileContext(nc) as tc:
        with (
            tc.tile_pool(name="const", bufs=1) as cpool,
            tc.tile_pool(name="stage", bufs=4) as spool,
            tc.tile_pool(name="psum", bufs=2, space=bass.MemorySpace.PSUM) as ppool,
            tc.tile_pool(name="mpsum", bufs=2, space=bass.MemorySpace.PSUM) as mps,
        ):
            # ---- tiles ----
            pa_t = cpool.tile([PA_ROWS, PA_COLS], bf16, tag="pa")
            ga_t = cpool.tile([GT_ROWS, GA_COLS], bf16, tag="ga")
            gb_t = cpool.tile([GT_ROWS, GB_COLS], bf16, tag="gb")
            e1 = cpool.tile([1, 8], f32, tag="e1")
            e1o = cpool.tile([1, 8], f32, tag="e1o")
            h3a = cpool.tile([GT_ROWS, HB], bf16, tag="h3a")
            h3b = cpool.tile([GT_ROWS, HB], bf16, tag="h3b")
            hts = {
                (l, c): cpool.tile(
                    [101, HB], bf16, tag=f"h{l}{c}", name=f"h{l}{c}"
                )
                for l in range(2)
                for c in range(2)
            }
            # ---- input loads. pa (3 fat descriptors) on sync. The MLP
            # weights + first half of G ride in one fat transfer (ga) that
            # lands first and unblocks L2 and the first psum chunks; the
            # rest of G (gb) follows. Thin per-tensor loads would serialize
            # ~20ns/descriptor on one DMA queue; extra DIRECT2D issues cost
            # ~0.7us each on the sequencer, so exactly two scalar issues ----
            nc.sync.dma_start(ga_t[:], ga_d[:])
            nc.scalar.dma_start(pa_t[:], pa_d[:])
            nc.sync.dma_start(gb_t[:], gb_d[:])

            # ---- preload the exp table set on ScalarE; init consts ----
            nc.vector.memset(e1[:], 0.0)
            nc.scalar.activation(e1o[:], e1[:], Exp)
            # ones rows for the bias/c folds. Engine APs must start at a
            # partition in {0,32,64,96}, so memset from 96; the relu writes
            # to [0:100] later overwrite rows 96..99 with the real values.
            nc.vector.memset(h3a[96:102, :], 1.0)
            nc.vector.memset(h3b[96:102, :], 1.0)
            for t in hts.values():
                nc.vector.memset(t[96:101, :], 1.0)

            w1 = pa_t[0:3, 512:612]          # row 2 = b1f
            w2 = ga_t[0:101, 0:100]          # row 100 = b2
            w3 = gb_t[0:101, 0:100]          # row 100 = b3

            # ---- MLP, two interleaved 256-wide chains; biases ride in the
            # matmuls via ones rows. Chain A relus on DVE, chain B on
            # ScalarE (same ACT table set as exp; GpSimd can't read PSUM)
            # so the chains' relus run concurrently. ----
            for lyr, wt in enumerate([w1, w2, w3]):
                for c in range(2):
                    cs = slice(c * HB, (c + 1) * HB)
                    src = pa_t[0:3, cs] if lyr == 0 else hts[(lyr - 1, c)][:]
                    ps = mps.tile([128, 256], f32, tag="mps")
                    nc.tensor.matmul(ps[0:100, 0:HB], wt, src)
                    dst = hts[(lyr, c)][0:100, :] if lyr < 2 else (
                        (h3a if c == 0 else h3b)[0:100, :]
                    )
                    if c == 0:
                        nc.vector.tensor_scalar(
                            dst, ps[0:100, 0:HB], 0.0, None, Max
                        )
                    else:
                        nc.scalar.activation(dst, ps[0:100, 0:HB], Relu)

            # ---- main: out[g] = exp(h3aug_g.T @ GTaug), one store per g.
            # G columns [off] come from ga (offset 200) for off<512, else
            # from gb. Group 0 runs a small 256-col first chunk + exp so
            # the ACT stream starts as early as possible; the last store
            # is issued from scalar itself (no cross-engine handoff). ----
            def grhs(off, w):
                if off < 512:
                    return ga_t[:, 100 + off : 100 + off + w]
                return gb_t[:, 100 + off - 512 : 100 + off - 512 + w]

            store_eng = [nc.sync, nc.sync, nc.sync, nc.scalar]
            for g in range(N_GRP):
                h3 = h3a if g < 2 else h3b
                lhsT = h3[:, (g % 2) * 128 : (g % 2 + 1) * 128]
                ps = ppool.tile([128, P_CORE], f32, tag="ps")
                stage = spool.tile([128, P_CORE], bf16, tag="stage")
                orow = out_d[g * 128 : (g + 1) * 128, :]
                if g == 0:
                    # first 256-col chunk goes to its OWN psum tile so the
                    # remaining chunk matmuls don't pick up a false WAR
                    # hazard against the first exp (psum hazards are
                    # tile-granular); the ACT stream starts right after
                    # this small first matmul
                    psa = mps.tile([128, 256], f32, tag="mps", name="psa")
                    nc.tensor.matmul(psa[:, 0:128], lhsT, grhs(0, 128))
                    nc.scalar.activation(stage[:, :128], psa[:, :128], Exp)
                    for off, w in [(128, 128), (256, 256), (512, 512), (1024, 32)]:
                        nc.tensor.matmul(
                            ps[:, off : off + w], lhsT, grhs(off, w)
                        )
                    nc.scalar.activation(
                        stage[:, 128:], ps[:, 128:P_CORE], Exp
                    )
                    store_eng[g].dma_start(orow, stage[:])
                elif g < N_GRP - 1:
                    for off in range(0, P_CORE, SUB):
                        w = min(SUB, P_CORE - off)
                        nc.tensor.matmul(
                            ps[:, off : off + w], lhsT, grhs(off, w)
                        )
                    nc.scalar.activation(stage[:], ps[:], Exp)
                    store_eng[g].dma_start(orow, stage[:])
                else:
                    for off in range(0, P_CORE, SUB):
                        w = min(SUB, P_CORE - off)
                        nc.tensor.matmul(
                            ps[:, off : off + w], lhsT, grhs(off, w)
                        )
                    # last group: store in two halves issued from two DGEs
                    # in parallel to shorten the end-of-kernel DMA tail
                    nc.scalar.activation(stage[:], ps[:], Exp)
                    nc.sync.dma_start(orow[:, 0:528], stage[:, 0:528])
                    nc.scalar.dma_start(orow[:, 528:], stage[:, 528:])

    nc.compile()
    return nc


def _get_cached():
    if "nc" not in _CACHE:
        _CACHE["nc"] = _build_program()
    if "consts" not in _CACHE:
        _CACHE["consts"] = _build_constants()
    return (_CACHE["nc"],) + _CACHE["consts"]


def _make_in_maps(phi, W1, b1, W2, b2, W3, b3, W4, b4, E64):
    # fold the input normalization into the first layer (f64 host math)
    scale = DPHI / SIG
    shift = (MIN_PHI - MU) / SIG
    W1_64 = np.asarray(W1, np.float64)
    W1f = W1_64 * scale[:, None]
    b1f = np.asarray(b1, np.float64) + shift @ W1_64

    pa = np.zeros((PA_ROWS, PA_COLS), ml_dtypes.bfloat16)
    pa[0:2, 0:512] = _bf16(np.asarray(phi, np.float64).T)
    pa[2, 0:512] = _bf16(1.0)                        # ones row (bias fold)
    pa[0:2, 512:612] = _bf16(W1f)
    pa[2, 512:612] = _bf16(b1f)


    # fold layer 4 into the spline-eval matrix: G (P,100), c (P,)
    G = E64 @ np.asarray(W4, np.float64).T          # (P_PAD, 100)
    c = E64 @ np.asarray(b4, np.float64) + np.log(np.float64(NORM))
    c_hi = _bf16(c)
    c_lo = _bf16(c - c_hi.astype(np.float64))
    GTb = _bf16(G.T)                                 # (100, P_PAD)

    in_maps = []
    for cix in range(N_CORES):
        p0 = cix * P_CORE
        ga = np.zeros((GT_ROWS, GA_COLS), ml_dtypes.bfloat16)
        ga[0:100, 0:100] = _bf16(W2)
        ga[100, 0:100] = _bf16(b2)
        ga[0:100, 100:612] = GTb[:, p0 : p0 + 512]
        ga[100, 100:612] = c_hi[p0 : p0 + 512]
        ga[101, 100:612] = c_lo[p0 : p0 + 512]
        gb = np.zeros((GT_ROWS, GB_COLS), ml_dtypes.bfloat16)
        gb[0:100, 0:100] = _bf16(W3)
        gb[100, 0:100] = _bf16(b3)
        gb[0:100, 100:] = GTb[:, p0 + 512 : p0 + P_CORE]
        gb[100, 100:] = c_hi[p0 + 512 : p0 + P_CORE]
        gb[101, 100:] = c_lo[p0 + 512 : p0 + P_CORE]
        in_maps.append({"pa": pa, "ga": ga, "gb": gb})
    return in_maps


def kernel(phi, W1, b1, W2, b2, W3, b3, W4, b4):
    from concourse.bass_utils import run_bass_kernel_spmd

    nc, E64, IDX = _get_cached()
    in_maps = _make_in_maps(phi, W1, b1, W2, b2, W3, b3, W4, b4, E64)
    res = run_bass_kernel_spmd(nc, in_maps, core_ids=list(range(N_CORES)))
    uniq = np.concatenate(
        [r["out"].astype(np.float32) for r in res.results], axis=1
    )  # (512, 8448) f32
    full = np.take(uniq, IDX, axis=1)  # (512, 65536) constant-gather replication
    return np.ascontiguousarray(full.reshape(B, 256, 256))


# revision 26
# speedup vs baseline: 1.1504x; 1.1504x over previous
"""Trainium2 Bass kernel for the CMB power-spectrum emulator problem.

Math: a 4-layer MLP maps phi (512,2) -> diag (128 knots, 512 ch); a natural
cubic spline through the 128 knots is evaluated on a constant 256x256
isotropic-frequency grid, then exp(.)*NORM.

Structural collapses (all input-independent or host-cheap):
 1. The spline is linear in the knot values: the whole spline stage is a
    constant matrix E (grid_points, 128) applied to the knot values.
 2. The grid has an exact 8-fold dihedral symmetry: only 8385 of the 65536
    grid points are distinct; the device computes the unique points and the
    host replicates them with a constant gather.
 3. The last (linear) MLP layer commutes with E: G = E @ W4.T (P,100) and
    c = E @ b4 + ln NORM are folded on the host, so the device runs only the
    3 relu layers and one (102-row) matmul per point block. The per-point
    bias c rides inside the matmul as two bf16 hi/lo rows multiplied by
    constant-1 rows of the stationary operand (bf16 alone would lose the
    ~9.45 magnitude of ln NORM; the hi/lo split restores ~16-bit precision).

Device work per core (point sharding, 1056 points, 512 channels, all bf16):
  junk matmuls warm the PE HAM clock-gate during the input-DMA window;
  a dummy exp preloads the ACT table set at t=0.
  MLP as two interleaved 256-wide chains -> h3 (100, 512) bf16 (+ ones rows)
  per 128-channel group g: psum = h3_g.T @ GT   (TensorE, bf16, K=102)
                           stage = exp(psum)    (ScalarE LUT)
                           store (128, 1056) bf16 (DMA, issue spread over
                           sync/gpsimd/vector queues)
Host: upcast bf16->f32, constant gather to (512, 256, 256).
"""

import os

import ml_dtypes
import numpy as np

B = 512
N_CORES = 8
N_UNIQ = 129 * 130 // 2       # 8385 distinct grid values
P_CORE = 1056                 # per-core unique points (8 x 1056 = 8448 padded)
P_PAD = N_CORES * P_CORE
NORM = 1.0 / 12661.0
HB = B // 2                   # per-chain batch width

MIN_PHI = np.array([50.0, 0.0075], np.float64)
DPHI = np.array([40.0, 0.0492], np.float64)
MU = np.array([70.0, 0.032], np.float64)
SIG = np.array([20.0, 0.025], np.float64)

PA_COLS = 612   # phiT (512) | W1f (100); row 2 = ones | b1f (bias fold)
PA_ROWS = 3
GT_ROWS = 102   # G.T (100) ; c_hi ; c_lo (+ ones rows in lhsT)
GA_COLS = 200 + 512   # W2aug | W3aug | G.T cols 0:512 (fat rows, lands first)
GB_COLS = P_CORE - 512  # G.T cols 512:1056

_CACHE = {}


def _bf16(x):
    return np.asarray(x, np.float64).astype(ml_dtypes.bfloat16)


def _spline_eval_matrix_f64(wn_vals):
    """E (len(wn_vals), 128) f64: natural-cubic-spline evaluation at wn_vals,
    linear in the 128 knot values (knots t_k = sqrt(2)*k in fp32)."""
    wn = (256.0 * np.fft.fftfreq(256, d=1.0)).reshape(256, 1)
    wn_iso = np.sqrt(wn**2 + wn.reshape(1, 256) ** 2)
    t32 = np.fft.fftshift(wn_iso).diagonal()[128:].astype(np.float32)  # (128,)

    n = 128
    t = t32.astype(np.float64)
    h = np.diff(t)
    A = np.diag(2.0 * (h[:-1] + h[1:])) + np.diag(h[1:-1], 1) + np.diag(h[1:-1], -1)
    D1 = np.zeros((n - 1, n))
    for i in range(n - 1):
        D1[i, i] = -1.0 / h[i]
        D1[i, i + 1] = 1.0 / h[i]
    D2 = 6.0 * (D1[1:] - D1[:-1])
    L = np.zeros((n, n))
    L[1:-1] = np.linalg.solve(A, D2)

    Sa = np.eye(n)[: n - 1]
    Sb = D1 - (h[:, None] / 6.0) * (2.0 * L[:-1] + L[1:])
    Sc = L[:-1] / 2.0
    Sd = (L[1:] - L[:-1]) / (6.0 * h[:, None])

    w32 = wn_vals.astype(np.float32)
    idx = np.clip(np.searchsorted(t32, w32, side="right") - 1, 0, n - 2)
    f = (w32 - t32[idx]).astype(np.float64)[:, None]
    return Sa[idx] + f * (Sb[idx] + f * (Sc[idx] + f * Sd[idx]))  # f64


def _build_constants():
    """E64 (P_PAD, 128) f64 spline-eval matrix at the unique points (zero
    rows for padding), and IDX (65536,) int32 full-grid -> unique column."""
    k = np.arange(256)
    absw = np.minimum(k, 256 - k)
    ai = np.minimum(absw[:, None], absw[None, :])
    bi = np.maximum(absw[:, None], absw[None, :])
    uid = (bi * (bi + 1)) // 2 + ai  # (256,256) in [0, N_UNIQ)

    bs = np.concatenate([np.full(b + 1, b) for b in range(129)])
    as_ = np.concatenate([np.arange(b + 1) for b in range(129)])
    wn_vals = np.sqrt(as_.astype(np.float64) ** 2 + bs.astype(np.float64) ** 2)

    E = np.zeros((P_PAD, 128), np.float64)
    E[:N_UNIQ] = _spline_eval_matrix_f64(wn_vals)
    return E, uid.ravel().astype(np.int32)


def _build_program():
    import concourse.bass as bass
    import concourse.bacc as bacc
    import concourse.mybir as mybir
    from concourse import tile

    f32 = mybir.dt.float32
    bf16 = mybir.dt.bfloat16
    nc = bacc.Bacc("TRN2", target_bir_lowering=False, debug=False)

    pa_d = nc.dram_tensor("pa", [PA_ROWS, PA_COLS], bf16, kind="ExternalInput")
    ga_d = nc.dram_tensor("ga", [GT_ROWS, GA_COLS], bf16, kind="ExternalInput")
    gb_d = nc.dram_tensor("gb", [GT_ROWS, GB_COLS], bf16, kind="ExternalInput")
    out_d = nc.dram_tensor("out", [B, P_CORE], bf16, kind="ExternalOutput")

    Exp = mybir.ActivationFunctionType.Exp
    Relu = mybir.ActivationFunctionType.Relu
    Max = mybir.AluOpType.max

    N_GRP = 4
    SUB = 512  # matmul free chunk (PSUM bank)

    with tile.TileContext(nc) as tc:
        with (
            tc.tile_pool(name="const", bufs=1) as cpool,
            tc.tile_pool(name="stage", bufs=4) as spool,
            tc.tile_pool(name="psum", bufs=2, space=bass.MemorySpace.PSUM) as ppool,
            tc.tile_pool(name="mpsum", bufs=2, space=bass.MemorySpace.PSUM) as mps,
        ):
            # ---- tiles ----
            pa_t = cpool.tile([PA_ROWS, PA_COLS], bf16, tag="pa")
            ga_t = cpool.tile([GT_ROWS, GA_COLS], bf16, tag="ga")
            gb_t = cpool.tile([GT_ROWS, GB_COLS], bf16, tag="gb")
            e1 = cpool.tile([1, 8], f32, tag="e1")
            e1o = cpool.tile([1, 8], f32, tag="e1o")
            h3a = cpool.tile([GT_ROWS, HB], bf16, tag="h3a")
            h3b = cpool.tile([GT_ROWS, HB], bf16, tag="h3b")
            hts = {
                (l, c): cpool.tile(
                    [101, HB], bf16, tag=f"h{l}{c}", name=f"h{l}{c}"
                )
                for l in range(2)
                for c in range(2)
            }
            # ---- input loads. pa (3 fat descriptors) on sync. The MLP
            # weights + first half of G ride in one fat transfer (ga) that
            # lands first and unblocks L2 and the first psum chunks; the
            # rest of G (gb) follows. Thin per-tensor loads would serialize
            # ~20ns/descriptor on one DMA queue; extra DIRECT2D issues cost
            # ~0.7us each on the sequencer, so exactly two scalar issues ----
            nc.sync.dma_start(ga_t[:], ga_d[:])
            nc.scalar.dma_start(pa_t[:], pa_d[:])
            nc.sync.dma_start(gb_t[:], gb_d[:])

            # ---- preload the exp table set on ScalarE; init consts ----
            nc.vector.memset(e1[:], 0.0)
            nc.scalar.activation(e1o[:], e1[:], Exp)
            # ones rows for the bias/c folds. Engine APs must start at a
            # partition in {0,32,64,96}, so memset from 96; the relu writes
            # to [0:100] later overwrite rows 96..99 with the real values.
            nc.vector.memset(h3a[96:102, :], 1.0)
            nc.vector.memset(h3b[96:102, :], 1.0)
            for t in hts.values():
                nc.vector.memset(t[96:101, :], 1.0)

            w1 = pa_t[0:3, 512:612]          # row 2 = b1f
            w2 = ga_t[0:101, 0:100]          # row 100 = b2
            w3 = ga_t[0:101, 100:200]        # row 100 = b3

            # ---- MLP, two interleaved 256-wide chains; biases ride in the
            # matmuls via ones rows. Chain A relus on DVE, chain B on
            # ScalarE (same ACT table set as exp; GpSimd can't read PSUM)
            # so the chains' relus run concurrently. ----
            for lyr, wt in enumerate([w1, w2, w3]):
                for c in range(2):
                    cs = slice(c * HB, (c + 1) * HB)
                    src = pa_t[0:3, cs] if lyr == 0 else hts[(lyr - 1, c)][:]
                    ps = mps.tile([128, 256], f32, tag="mps")
                    nc.tensor.matmul(ps[0:100, 0:HB], wt, src)
                    dst = hts[(lyr, c)][0:100, :] if lyr < 2 else (
                        (h3a if c == 0 else h3b)[0:100, :]
                    )
                    if c == 0:
                        nc.vector.tensor_scalar(
                            dst, ps[0:100, 0:HB], 0.0, None, Max
                        )
                    else:
                        nc.scalar.activation(dst, ps[0:100, 0:HB], Relu)

            # ---- main: out[g] = exp(h3aug_g.T @ GTaug), one store per g.
            # G columns [off] come from ga (offset 200) for off<512, else
            # from gb. Group 0 runs a small 256-col first chunk + exp so
            # the ACT stream starts as early as possible; the last store
            # is issued from scalar itself (no cross-engine handoff). ----
            def grhs(off, w):
                if off < 512:
                    return ga_t[:, 200 + off : 200 + off + w]
                return gb_t[:, off - 512 : off - 512 + w]

            store_eng = [nc.sync, nc.sync, nc.sync, nc.scalar]
            for g in range(N_GRP):
                h3 = h3a if g < 2 else h3b
                lhsT = h3[:, (g % 2) * 128 : (g % 2 + 1) * 128]
                ps = ppool.tile([128, P_CORE], f32, tag="ps")
                stage = spool.tile([128, P_CORE], bf16, tag="stage")
                orow = out_d[g * 128 : (g + 1) * 128, :]
                if g == 0:
                    # first 256-col chunk goes to its OWN psum tile so the
                    # remaining chunk matmuls don't pick up a false WAR
                    # hazard against the first exp (psum hazards are
                    # tile-granular); the ACT stream starts right after
                    # this small first matmul
                    psa = mps.tile([128, 256], f32, tag="mps", name="psa")
                    nc.tensor.matmul(psa[:, 0:256], lhsT, grhs(0, 256))
                    nc.scalar.activation(stage[:, :256], psa[:, :256], Exp)
                    for off, w in [(256, 256), (512, 512), (1024, 32)]:
                        nc.tensor.matmul(
                            ps[:, off : off + w], lhsT, grhs(off, w)
                        )
                    nc.scalar.activation(
                        stage[:, 256:], ps[:, 256:P_CORE], Exp
                    )
                    store_eng[g].dma_start(orow, stage[:])
                elif g < N_GRP - 1:
                    for off in range(0, P_CORE, SUB):
                        w = min(SUB, P_CORE - off)
                        nc.tensor.matmul(
                            ps[:, off : off + w], lhsT, grhs(off, w)
                        )
                    nc.scalar.activation(stage[:], ps[:], Exp)
                    store_eng[g].dma_start(orow, stage[:])
                else:
                    for off in range(0, P_CORE, SUB):
                        w = min(SUB, P_CORE - off)
                        nc.tensor.matmul(
                            ps[:, off : off + w], lhsT, grhs(off, w)
                        )
                    # last group: store in two halves issued from two DGEs
                    # in parallel to shorten the end-of-kernel DMA tail
                    nc.scalar.activation(stage[:], ps[:], Exp)
                    nc.sync.dma_start(orow[:, 0:528], stage[:, 0:528])
                    nc.scalar.dma_start(orow[:, 528:], stage[:, 528:])

    nc.compile()
    return nc


def _get_cached():
    if "nc" not in _CACHE:
        _CACHE["nc"] = _build_program()
    if "consts" not in _CACHE:
        _CACHE["consts"] = _build_constants()
    return (_CACHE["nc"],) + _CACHE["consts"]


def _make_in_maps(phi, W1, b1, W2, b2, W3, b3, W4, b4, E64):
    # fold the input normalization into the first layer (f64 host math)
    scale = DPHI / SIG
    shift = (MIN_PHI - MU) / SIG
    W1_64 = np.asarray(W1, np.float64)
    W1f = W1_64 * scale[:, None]
    b1f = np.asarray(b1, np.float64) + shift @ W1_64

    pa = np.zeros((PA_ROWS, PA_COLS), ml_dtypes.bfloat16)
    pa[0:2, 0:512] = _bf16(np.asarray(phi, np.float64).T)
    pa[2, 0:512] = _bf16(1.0)                        # ones row (bias fold)
    pa[0:2, 512:612] = _bf16(W1f)
    pa[2, 512:612] = _bf16(b1f)


    # fold layer 4 into the spline-eval matrix: G (P,100), c (P,)
    G = E64 @ np.asarray(W4, np.float64).T          # (P_PAD, 100)
    c = E64 @ np.asarray(b4, np.float64) + np.log(np.float64(NORM))
    c_hi = _bf16(c)
    c_lo = _bf16(c - c_hi.astype(np.float64))
    GTb = _bf16(G.T)                                 # (100, P_PAD)

    in_maps = []
    for cix in range(N_CORES):
        p0 = cix * P_CORE
        ga = np.zeros((GT_ROWS, GA_COLS), ml_dtypes.bfloat16)
        ga[0:100, 0:100] = _bf16(W2)
        ga[0:100, 100:200] = _bf16(W3)
        ga[100, 0:100] = _bf16(b2)
        ga[100, 100:200] = _bf16(b3)
        ga[0:100, 200:712] = GTb[:, p0 : p0 + 512]
        ga[100, 200:712] = c_hi[p0 : p0 + 512]
        ga[101, 200:712] = c_lo[p0 : p0 + 512]
        gb = np.zeros((GT_ROWS, GB_COLS), ml_dtypes.bfloat16)
        gb[0:100] = GTb[:, p0 + 512 : p0 + P_CORE]
        gb[100] = c_hi[p0 + 512 : p0 + P_CORE]
        gb[101] = c_lo[p0 + 512 : p0 + P_CORE]
        in_maps.append({"pa": pa, "ga": ga, "gb": gb})
    return in_maps


def kernel(phi, W1, b1, W2, b2, W3, b3, W4, b4):
    from concourse.bass_utils import run_bass_kernel_spmd

    nc, E64, IDX = _get_cached()
    in_maps = _make_in_maps(phi, W1, b1, W2, b2, W3, b3, W4, b4, E64)
    res = run_bass_kernel_spmd(nc, in_maps, core_ids=list(range(N_CORES)))
    uniq = np.concatenate(
        [r["out"].astype(np.float32) for r in res.results], axis=1
    )  # (512, 8448) f32
    full = np.take(uniq, IDX, axis=1)  # (512, 65536) constant-gather replication
    return np.ascontiguousarray(full.reshape(B, 256, 256))
